# revision 8
# baseline (speedup 1.0000x reference)
"""Multi-head causal attention on 8 Trainium2 NeuronCores.

Problem: x [2, 2048, 1024] f32; Wq/Wk/Wv [1024, 1024]; Wo [1024, 1024]; bo [1024].
  q/k/v = split_heads(x @ W*)  (16 heads, head_dim 64)
  scores = q k^T, causal mask, / sqrt(1024), softmax, out = (w v) @ Wo + bo

Sharding: tensor-parallel over heads, 8-way (Megatron-style): core c computes
heads {2c, 2c+1} for BOTH batches. The concat+out_proj needs all heads, so
cores exchange attention outputs with a single 8-rank AllToAll (each core
sends, per destination core j, its two heads' attn^T restricted to j's output
row-slice). After the exchange core j holds attn^T [1024, 512] for
(batch j//4, rows 512*(j%4):+512), applies the full out_proj + bias, and
returns its 512-row slice of the output; the host reassembles.

On-chip layout trick: attention is computed fully transposed
(scores^T = K Q^T in [k, q] layout) so the softmax weights come out exactly in
the layout the attn-value matmul wants as its moving operand, and the AV
result comes out as attn^T [d, q] which is exactly the stationary layout
out_proj wants. Zero on-chip transposes in the attention path. The softmax
denominator is obtained for free by augmenting V with a ones-column (row 64 of
the AV psum accumulates sum(w)).

Compute dtype bf16 (fp32 accumulation in PSUM).
"""

from contextlib import ExitStack

import numpy as np

import concourse.bass as bass
import concourse.tile as tile
from concourse import bacc, mybir
from concourse.bass_utils import run_bass_kernel_spmd
from concourse.masks import make_identity

F32 = mybir.dt.float32
BF16 = mybir.dt.bfloat16

N_CORES = 8
B = 2
S = 2048
D = 1024
H = 16
DH = 64
H_PER = 2              # heads per core
DCOL = H_PER * DH      # 128: projection output cols per core
KT = D // 128          # 8 contraction tiles
SB = S // 128          # 16 sequence blocks
NQS = S // 512         # 4 q-spans
S_SLICE = S // 4       # 512 output rows per core
SCALE = 1.0 / np.sqrt(np.float32(D))

_CACHE = {}


def build():
    """Build the SPMD program (identical on all 8 cores)."""
    nc = bacc.Bacc("TRN2", target_bir_lowering=False, debug=False)

    x_t = nc.dram_tensor("x", [B, S, D], F32, kind="ExternalInput")
    wq_t = nc.dram_tensor("wq", [D, DCOL], F32, kind="ExternalInput")
    wk_t = nc.dram_tensor("wk", [D, DCOL], F32, kind="ExternalInput")
    wv_t = nc.dram_tensor("wv", [D, DCOL], F32, kind="ExternalInput")
    wo_t = nc.dram_tensor("wo", [D, D], F32, kind="ExternalInput")
    bo_t = nc.dram_tensor("bo", [1, D], F32, kind="ExternalInput")
    out_t = nc.dram_tensor("out", [S_SLICE, D], F32, kind="ExternalOutput")

    # collective buffers (internal DRAM)
    warm_in = nc.dram_tensor("warm_in", [8, 16], F32)
    warm_out = nc.dram_tensor("warm_out", [8, 16], F32)
    a2a_in = nc.dram_tensor("a2a_in", [8, 128, 512], BF16)
    a2a_out = nc.dram_tensor("a2a_out", [8, 128, 512], BF16)

    with tile.TileContext(nc) as tc, ExitStack() as ctx:
        const = ctx.enter_context(tc.tile_pool(name="const", bufs=1))
        persist = ctx.enter_context(tc.tile_pool(name="persist", bufs=1))
        stage = ctx.enter_context(tc.tile_pool(name="stage", bufs=3))
        wstage = ctx.enter_context(tc.tile_pool(name="wstage", bufs=2))
        wpool = ctx.enter_context(tc.tile_pool(name="wpool", bufs=4))
        rbpool = ctx.enter_context(tc.tile_pool(name="rbpool", bufs=3))
        spool = ctx.enter_context(tc.tile_pool(name="spool", bufs=4))
        opool = ctx.enter_context(tc.tile_pool(name="opool", bufs=3))
        ps_mm = ctx.enter_context(tc.tile_pool(name="ps_mm", bufs=3, space="PSUM"))
        ps_o = ctx.enter_context(tc.tile_pool(name="ps_o", bufs=2, space="PSUM"))
        ps_t = ctx.enter_context(tc.tile_pool(name="ps_t", bufs=2, space="PSUM"))

        # ---- warmup collective: absorbs the per-execution ncfw entry cost
        # concurrently with the compute phase.
        nc.gpsimd.collective_compute(
            "AllToAll", mybir.AluOpType.bypass,
            replica_groups=[list(range(8))],
            ins=[warm_in.ap().opt()], outs=[warm_out.ap().opt()],
        )

        identity = const.tile([128, 128], BF16)
        make_identity(nc, identity)

        # ---- load weights (once) -------------------------------------------
        def load_cast(dram_ap, kt_cols, name):
            """DRAM [1024, C] f32 -> SBUF [128, KT, C] bf16."""
            st = wstage.tile([128, KT, kt_cols], F32, tag="wst" + name[:1])
            nc.sync.dma_start(
                out=st, in_=dram_ap.rearrange("(kt p) c -> p kt c", p=128))
            bf = persist.tile([128, KT, kt_cols], BF16, tag=name)
            nc.vector.tensor_copy(out=bf, in_=st)
            return bf

        wq_bf = load_cast(wq_t[:, :], DCOL, "wq")
        wk_bf = load_cast(wk_t[:, :], DCOL, "wk")
        wv_bf = load_cast(wv_t[:, :], DCOL, "wv")
        # Wo staged per-kt to limit staging SBUF
        wo_bf = persist.tile([128, KT, D], BF16, tag="wo")
        for kt in range(KT):
            st = wstage.tile([128, D], F32, tag="wost")
            nc.sync.dma_start(out=st, in_=wo_t[kt * 128:(kt + 1) * 128, :])
            nc.vector.tensor_copy(out=wo_bf[:, kt, :], in_=st)
        bias_b = persist.tile([128, D], F32, tag="bias")
        nc.sync.dma_start(out=bias_b, in_=bo_t[0:1, :].to_broadcast([128, D]))

        attnT = [
            persist.tile([128, S], BF16, tag=f"attnT{b}", name=f"attnT{b}")
            for b in range(B)
        ]

        for b in range(B):
            # ---- x^T (bf16) via PE transpose -------------------------------
            xT = persist.tile([128, KT, S], BF16, tag="xT")  # reused across b
            for sb in range(SB):
                xn = stage.tile([128, D], F32, tag="xn")
                nc.sync.dma_start(out=xn, in_=x_t[b, sb * 128:(sb + 1) * 128, :])
                xb = stage.tile([128, D], BF16, tag="xb")
                nc.vector.tensor_copy(out=xb, in_=xn)
                for g in range(2):  # 2 groups of 4 d-blocks -> one psum bank
                    pt = ps_t.tile([128, 4, 128], BF16, tag="pt")
                    for k in range(4):
                        kt = g * 4 + k
                        nc.tensor.transpose(
                            pt[:, k, :], xb[:, kt * 128:(kt + 1) * 128], identity)
                    dst = xT[:, g * 4:(g + 1) * 4, sb * 128:(sb + 1) * 128]
                    nc.vector.tensor_copy(out=dst, in_=pt)

            # ---- projections ----------------------------------------------
            # Q^T, K^T: [128 rows(=2 heads x 64), S]
            qT = persist.tile([128, S], BF16, tag="qT")
            kTt = persist.tile([128, S], BF16, tag="kT")
            for w_bf, dest in ((wq_bf, qT), (wk_bf, kTt)):
                for nt in range(NQS):
                    ps = ps_mm.tile([128, 512], F32, tag="mm")
                    for kt in range(KT):
                        nc.tensor.matmul(
                            ps, lhsT=w_bf[:, kt, :],
                            rhs=xT[:, kt, nt * 512:(nt + 1) * 512],
                            start=(kt == 0), stop=(kt == KT - 1))
                    nc.scalar.copy(dest[:, nt * 512:(nt + 1) * 512], ps)
            # V' natural [s, 2*(64+1)] with ones columns
            vp = persist.tile([128, SB, H_PER * (DH + 1)], BF16, tag="vp")
            ones_view = vp.rearrange("p s (h c) -> p s h c", c=DH + 1)[:, :, :, DH:]
            nc.vector.memset(ones_view, 1.0)
            for sb in range(SB):
                ps = ps_mm.tile([128, DCOL], F32, tag="mm")
                for kt in range(KT):
                    nc.tensor.matmul(
                        ps, lhsT=xT[:, kt, sb * 128:(sb + 1) * 128],
                        rhs=wv_bf[:, kt, :],
                        start=(kt == 0), stop=(kt == KT - 1))
                dst = vp.rearrange("p s (h c) -> p s h c", c=DH + 1)[:, sb, :, :DH]
                nc.scalar.copy(dst, ps.rearrange("p (h c) -> p h c", c=DH))

            # ---- attention (transposed layout) -----------------------------
            for h in range(H_PER):
                hr = h * DH
                for qs in range(NQS):
                    nkb = 4 * qs + 4
                    o_ps = ps_o.tile([DH + 1, 512], F32, tag="o")
                    for kb in range(nkb):
                        s_ps = ps_mm.tile([128, 512], F32, tag="mm")
                        nc.tensor.matmul(
                            s_ps,
                            lhsT=kTt[hr:hr + DH, kb * 128:(kb + 1) * 128],
                            rhs=qT[hr:hr + DH, qs * 512:(qs + 1) * 512],
                            start=True, stop=True)
                        w_bf_t = wpool.tile([128, 512], BF16, tag="w")
                        nc.scalar.activation(
                            w_bf_t, s_ps, mybir.ActivationFunctionType.Exp,
                            scale=float(SCALE))
                        if kb >= 4 * qs:
                            # causal: keep iff (512qs + f) - (128kb + p) >= 0
                            nc.gpsimd.affine_select(
                                out=w_bf_t, in_=w_bf_t,
                                pattern=[[1, 512]],
                                compare_op=mybir.AluOpType.is_ge,
                                fill=0.0,
                                base=512 * qs - 128 * kb,
                                channel_multiplier=-1)
                        nc.tensor.matmul(
                            o_ps,
                            lhsT=vp[:, kb, h * (DH + 1):(h + 1) * (DH + 1)],
                            rhs=w_bf_t,
                            start=(kb == 0), stop=(kb == nkb - 1))
                    denom = spool.tile([1, 512], F32, tag="den")
                    nc.scalar.copy(denom, o_ps[DH:DH + 1, :])
                    recip = spool.tile([1, 512], F32, tag="rec")
                    nc.vector.reciprocal(recip, denom)
                    rb = rbpool.tile([DH, 512], F32, tag="rb")
                    nc.gpsimd.partition_broadcast(rb, recip)
                    nc.vector.tensor_mul(
                        attnT[b][hr:hr + DH, qs * 512:(qs + 1) * 512],
                        o_ps[0:DH, :], rb)

            # ship this batch's attn^T chunks to the A2A input buffer:
            # chunk j=4b+jj <- attnT_b[:, 512jj:+512]
            nc.sync.dma_start(
                out=a2a_in[4 * b:4 * (b + 1)].rearrange("j p c -> p j c"),
                in_=attnT[b].rearrange("p (j c) -> p j c", c=512))

        # ---- exchange ------------------------------------------------------
        nc.gpsimd.collective_compute(
            "AllToAll", mybir.AluOpType.bypass,
            replica_groups=[list(range(8))],
            ins=[a2a_in.ap().opt()], outs=[a2a_out.ap().opt()],
        )

        # ---- out_proj on this core's [1024, 512] gathered attn^T -----------
        g_sb = persist.tile([128, KT, 512], BF16, tag="g")
        nc.sync.dma_start(
            out=g_sb,
            in_=a2a_out.ap().rearrange("kt p c -> p kt c"))
        for sb in range(4):
            for nt in range(2):
                ps = ps_mm.tile([128, 512], F32, tag="mm")
                for kt in range(KT):
                    nc.tensor.matmul(
                        ps, lhsT=g_sb[:, kt, sb * 128:(sb + 1) * 128],
                        rhs=wo_bf[:, kt, nt * 512:(nt + 1) * 512],
                        start=(kt == 0), stop=(kt == KT - 1))
                ot = opool.tile([128, 512], F32, tag="ot")
                nc.vector.tensor_add(ot, ps, bias_b[:, nt * 512:(nt + 1) * 512])
                nc.sync.dma_start(
                    out=out_t[sb * 128:(sb + 1) * 128, nt * 512:(nt + 1) * 512],
                    in_=ot)

    nc.compile()
    return nc


def shard_inputs(x, Wq, Wk, Wv, Wo, bo):
    """Full inputs -> per-core in_maps."""
    x = np.ascontiguousarray(np.asarray(x, dtype=np.float32))
    Wq = np.asarray(Wq, dtype=np.float32)
    Wk = np.asarray(Wk, dtype=np.float32)
    Wv = np.asarray(Wv, dtype=np.float32)
    Wo = np.ascontiguousarray(np.asarray(Wo, dtype=np.float32))
    bo = np.asarray(bo, dtype=np.float32).reshape(1, D)
    in_maps = []
    for c in range(N_CORES):
        cols = slice(c * DCOL, (c + 1) * DCOL)
        in_maps.append({
            "x": x,
            "wq": np.ascontiguousarray(Wq[:, cols]),
            "wk": np.ascontiguousarray(Wk[:, cols]),
            "wv": np.ascontiguousarray(Wv[:, cols]),
            "wo": Wo,
            "bo": bo,
        })
    return in_maps


def assemble_output(results):
    """Per-core out slices -> full [B, S, D]."""
    out = np.empty((B, S, D), dtype=np.float32)
    for c in range(N_CORES):
        b, sl = c // 4, c % 4
        out[b, sl * S_SLICE:(sl + 1) * S_SLICE, :] = results[c]["out"]
    return out


def kernel(x, Wq, Wk, Wv, Wo, bo):
    if "nc" not in _CACHE:
        _CACHE["nc"] = build()
    nc = _CACHE["nc"]
    in_maps = shard_inputs(x, Wq, Wk, Wv, Wo, bo)
    res = run_bass_kernel_spmd(nc, in_maps, core_ids=list(range(N_CORES)))
    return assemble_output(res.results)
